# revision 2
# baseline (speedup 1.0000x reference)
"""Trainium2 Bass kernel for nn_Convolution_1176821039249.

Computes out = base_map * mean_k box_k(x) for k in {3,5,7,9,11,13,15} with
replicate padding, on 8 NeuronCores arranged as a 4x2 (rows x cols) grid:
each core owns a 1024-row x 2048-col block with a 7-pixel halo.

Algorithm (per core):
  The total 2D kernel K(di,dj) = sum_k 1/(7k^2) * 1[|di|<=k//2] 1[|dj|<=k//2]
  is decomposed over the horizontal "wing" basis
      T_0 = x(center),  T_m(j) = x(j-m) + x(j+m)   (m = 1..7)
  so that  out = sum_{b=0..7} P_b-vertical-band applied to T_b, where
      P_b(d) = sum_{k: k//2 >= max(b,|d|)} 1/(7k^2).
  Wings are fp16 tensor_tensor adds on DVE (2x mode); the vertical
  pyramid bands are 8 PSUM-accumulated banded matmuls on the PE per
  512-col chunk; ACT drains PSUM, GPSIMD multiplies by base_map.

  The 1024-row shard needs 9 partition tiles (8x114 + 112) vs 10 for the
  previous 512x4096 sharding (whose 56-row runt tile cost as much DVE/PE
  as a full tile -- both engines' cost is free-size only).  Wings are
  emitted in 512-col chunks and matmuls run chunk-outer/band-inner so the
  PE starts a few us into the kernel and never idles long enough to drop
  out of its max p-state.
"""

import numpy as np

F16 = np.float16

H = W = 4096
PAD = 7
N_CORES = 8
GRID_R, GRID_C = 4, 2
RPC = H // GRID_R           # 1024 rows per core
CPC = W // GRID_C           # 2048 cols per core
TILE_M = 114                # output rows per row tile (128 - 2*PAD)
N_TILES = 9                 # 8 * 114 + 112 = 1024
LAST_M = RPC - (N_TILES - 1) * TILE_M   # 112
CHUNK = 512                 # matmul N chunk (one PSUM bank of fp32)
N_CHUNKS = CPC // CHUNK     # 4
KERNEL_SIZES = (3, 5, 7, 9, 11, 13, 15)

_CACHE = {}


def _bands_np() -> np.ndarray:
    """lhsT band matrices, [128, 8*TILE_M] fp16.

    Band b column i row p holds P_b(p - i - 7): the vertical pyramid profile
    applied to wing tensor T_b.
    """
    w = {k: 1.0 / (7.0 * k * k) for k in KERNEL_SIZES}
    P = np.zeros((8, 15), dtype=np.float64)
    for b in range(8):
        for d in range(-7, 8):
            P[b, d + 7] = sum(w[k] for k in KERNEL_SIZES if k // 2 >= max(b, abs(d)))
    M = np.zeros((128, 8 * TILE_M), dtype=np.float64)
    for b in range(8):
        for i in range(TILE_M):
            p_lo = i  # d = p - i - 7 in [-7, 7]; P is indexed at d + 7 = p - i
            for p in range(p_lo, p_lo + 15):
                M[p, b * TILE_M + i] = P[b, p - i]
    return M.astype(F16)


# wing plane layout in the w tile: [m1, m3, m2, m4, m6, m5, m7]
_PLANE_OF = {1: 0, 3: 1, 2: 2, 4: 3, 6: 4, 5: 5, 7: 6}
# band emission order: rhs-ready order (center first, then wings as the
# fused fan adds that produce them complete)
_BAND_ORDER = (0, 1, 3, 2, 4, 6, 5, 7)


def _build_nc():
    import concourse.bass as bass
    import concourse.mybir as mybir
    import concourse.tile as tile

    dt = mybir.dt
    SHARD_R = RPC + 2 * PAD     # 1038
    SHARD_C = CPC + 2 * PAD     # 2062

    nc = bass.Bass()
    xb_d = nc.declare_dram_parameter("xb", [SHARD_R, SHARD_C], dt.float16, isOutput=False)
    base_d = nc.declare_dram_parameter("base", [RPC, CPC], dt.float16, isOutput=False)
    bands_d = nc.declare_dram_parameter("bands", [128, 8 * TILE_M], dt.float16, isOutput=False)
    out_d = nc.declare_dram_parameter("out", [RPC, CPC], dt.float32, isOutput=True)

    with tile.TileContext(nc) as tc:
        with (
            tc.tile_pool(name="const", bufs=1) as constp,
            tc.tile_pool(name="xin", bufs=2) as xpool,
            tc.tile_pool(name="wings", bufs=3) as wpool,
            tc.tile_pool(name="io", bufs=2) as iopool,
            tc.tile_pool(name="acc", bufs=6) as accpool,
            tc.tile_pool(name="psum", bufs=2, space="PSUM") as psump,
        ):
            bands_sb = constp.tile([128, 8 * TILE_M], dt.float16, name="bands_sb")
            nc.sync.dma_start(bands_sb[:], bands_d[:])

            for t in range(N_TILES):
                M = TILE_M if t < N_TILES - 1 else LAST_M
                K = M + 2 * PAD
                r0 = t * TILE_M
                # Dedicated slot per row tile: no slot reuse, so the load
                # DMAs carry no sync waits (walrus 1-wait DMA limit).
                xt = xpool.tile([128, SHARD_C], dt.float16, tag="xt", name="xt", bufs=N_TILES)
                nc.sync.dma_start(xt[:K, :], xb_d[r0:r0 + K, :])
                bt = iopool.tile([128, CPC], dt.float16, tag="bt", name="bt", bufs=N_TILES)
                nc.sync.dma_start(bt[:M, :], base_d[r0:r0 + M, :])

                def fan(src, start, n, step):
                    # [K, n, CHUNK] view: plane i starts at column
                    # start + i*step (overlapping windows; innermost stays
                    # stride-1 so the fp16 2x_1p DVE mode is preserved)
                    v = src[:K, start:start + CHUNK].unsqueeze(1)
                    lst = v.ap
                    lst[1] = (step, n)
                    v.ap = lst
                    return v

                # Fused wing adds per 512-col chunk: wings (1,3), (2,4,6) and
                # (5,7) each collapse into one DVE tensor_tensor via 3D fan
                # views (column offsets form stride -2/+2 arithmetic
                # sequences).  Chunked so tile 0's first matmuls start after
                # ~2us of DVE instead of a full tile of wings.
                w = wpool.tile([128, 7, CPC], dt.float16, tag="w", name="w")
                for q in range(N_CHUNKS):
                    c0 = q * CHUNK
                    nc.vector.tensor_add(w[:K, 0:2, c0:c0 + CHUNK],
                                         fan(xt, c0 + 6, 2, -2), fan(xt, c0 + 8, 2, 2))
                    nc.vector.tensor_add(w[:K, 2:5, c0:c0 + CHUNK],
                                         fan(xt, c0 + 5, 3, -2), fan(xt, c0 + 9, 3, 2))
                    nc.vector.tensor_add(w[:K, 5:7, c0:c0 + CHUNK],
                                         fan(xt, c0 + 2, 2, -2), fan(xt, c0 + 12, 2, 2))

                ps = psump.tile([128, CPC], dt.float32, tag="ps", name="ps")
                for q in range(N_CHUNKS):
                    cc = slice(q * CHUNK, (q + 1) * CHUNK)
                    for i, b in enumerate(_BAND_ORDER):
                        if b == 0:
                            rhs = xt[:K, PAD + q * CHUNK:PAD + q * CHUNK + CHUNK]
                        else:
                            rhs = w[:K, _PLANE_OF[b], cc]
                        nc.tensor.matmul(
                            ps[:M, cc],
                            bands_sb[:K, b * TILE_M:b * TILE_M + M],
                            rhs,
                            start=(i == 0),
                            stop=(i == 7),
                        )
                    # chunked drain+mul+store pipelines with the remaining
                    # chunks' matmuls (ACT drains PSUM; Pool cannot read PSUM)
                    acc = accpool.tile([128, CHUNK], dt.float32, tag="acc", name="acc")
                    nc.scalar.copy(acc[:M, :], ps[:M, cc])
                    nc.gpsimd.tensor_mul(acc[:M, :], acc[:M, :], bt[:M, cc])
                    nc.sync.dma_start(out_d[r0:r0 + M, cc], acc[:M, :])
    return nc


def _split_sync_waits(nc):
    """Walrus codegen only supports one sync wait per instruction; hoist
    extra waits onto injected NoOps on the instruction's engine (identical
    semantics: the sequencer blocks at the NoOp first, then at the
    instruction).  DMA instructions are issued from their engine's
    sequencer stream, so the same hoisting applies to them.
    """
    import concourse.mybir as mybir

    n_nops = 0
    for fn in nc.m.functions:
        for bb in fn.blocks:
            new = []
            for inst in bb.instructions:
                si = inst.sync_info
                if si is not None and si.on_wait and len(si.on_wait) > 1:
                    waits = list(si.on_wait)
                    hoist, keep = waits[:-1], waits[-1:]
                    for w in hoist:
                        nop = mybir.InstNoOp(name=f"{inst.name}-w{n_nops}", ins=[], outs=[])
                        nop.engine = inst.engine
                        nop.sync_info = mybir.SyncInfo(on_wait=[w], on_update=[])
                        new.append(nop)
                        n_nops += 1
                    if hoist:
                        inst.sync_info = mybir.SyncInfo(
                            on_wait=keep, on_update=list(si.on_update))
                new.append(inst)
            bb.instructions = new
    return n_nops


def _get_nc():
    if "nc" not in _CACHE:
        nc = _build_nc()
        _split_sync_waits(nc)
        _CACHE["nc"] = nc
    return _CACHE["nc"]


def _run(x: np.ndarray, base_map: np.ndarray, trace: bool = False):
    from concourse.bass_utils import run_bass_kernel_spmd

    nc = _get_nc()
    xp = np.pad(np.asarray(x, dtype=np.float32), PAD, mode="edge").astype(F16)
    base_map = np.ascontiguousarray(np.asarray(base_map, dtype=np.float32).astype(F16))
    bands = _bands_np()
    in_maps = []
    for c in range(N_CORES):
        rb, cb = divmod(c, GRID_C)
        r0, c0 = rb * RPC, cb * CPC
        in_maps.append({
            "xb": np.ascontiguousarray(xp[r0:r0 + RPC + 2 * PAD, c0:c0 + CPC + 2 * PAD]),
            "base": np.ascontiguousarray(base_map[r0:r0 + RPC, c0:c0 + CPC]),
            "bands": bands,
        })
    res = run_bass_kernel_spmd(nc, in_maps, list(range(N_CORES)), trace=trace)
    out = np.empty((H, W), dtype=np.float32)
    for c in range(N_CORES):
        rb, cb = divmod(c, GRID_C)
        r0, c0 = rb * RPC, cb * CPC
        out[r0:r0 + RPC, c0:c0 + CPC] = res.results[c]["out"]
    return out[None, None], res


def kernel(x: np.ndarray, base_map: np.ndarray) -> np.ndarray:
    out, _ = _run(x, base_map, trace=False)
    return out
